# revision 1
# baseline (speedup 1.0000x reference)
"""Trainium2 Bass kernel for nn_AdaptiveDecoder (shared MLP + hard-routed type heads).

Strategy:
  * Host: sort nodes by type; pad each type's count to a multiple of 8*128 and
    split evenly over 8 cores -> every core sees the SAME static layout of
    type-pure 128-row tiles, so the compiled SPMD program bakes in the
    tile->head mapping and the device does zero routing work.
  * Device: keep activations transposed ([feature, nodes]) the whole way so
    the three matmul stages chain without transposes.  LayerNorm's gamma/beta
    are folded into the head weights on the host; the per-node mean/std terms
    enter via K=1 rank-1 accumulation matmuls and one K=1 broadcast matmul.
  * Matmuls run as float32r (full-rate fp32 path on TRN2 TensorE).
"""

import sys

sys.path.insert(0, "/opt/trn_rl_repo")

from contextlib import ExitStack

import numpy as np

N_CORES = 8
LATENT, HIDDEN, OUT, TYPES = 512, 1024, 256, 3
P = 128
NB = 512  # node columns per block (moving-dim max for 4-byte matmul)
KL = LATENT // P  # 4 k-tiles, stage 1
KH = HIDDEN // P  # 8 k-tiles, stage 2 / head
MH = HIDDEN // P  # 8 m-chunks of hidden
MO = OUT // P  # 2 m-chunks of head output
LN_EPS = 1e-5


def build_program(blocks, R, use_c1=True, mm_bf16=False):
    """blocks: list of (type_idx, col_offset, n_cols); R: node columns per core."""
    import concourse.mybir as mybir
    import concourse.tile as tile
    from concourse import bacc, bass_isa

    dt = mybir.dt
    f32, f32r, bf16 = dt.float32, dt.float32r, dt.bfloat16
    mmdt = bf16 if mm_bf16 else f32r  # main matmul datapath dtype
    AF = mybir.ActivationFunctionType

    nc = bacc.Bacc("TRN2", target_bir_lowering=False, debug=False, num_devices=N_CORES)

    xt = nc.dram_tensor("xt", [LATENT, R], mmdt, kind="ExternalInput").ap()
    w1d = nc.dram_tensor("w1", [LATENT, HIDDEN], mmdt, kind="ExternalInput").ap()
    w2d = nc.dram_tensor("w2", [HIDDEN, HIDDEN], mmdt, kind="ExternalInput").ap()
    b1d = nc.dram_tensor("b1r", [P, MH], f32, kind="ExternalInput").ap()
    b2d = nc.dram_tensor("b2r", [P, MH], f32, kind="ExternalInput").ap()
    whpd = nc.dram_tensor("whp", [TYPES, HIDDEN, OUT], mmdt, kind="ExternalInput").ap()
    c1d = nc.dram_tensor("c1", [TYPES, 1, OUT], mmdt, kind="ExternalInput").ap()
    c2d = nc.dram_tensor("c2", [TYPES, 1, OUT], mmdt, kind="ExternalInput").ap()
    orowd = nc.dram_tensor("orow", [1, P], f32r, kind="ExternalInput").ap()
    outd = nc.dram_tensor("out", [OUT, R], f32, kind="ExternalOutput").ap()

    def cv(ap):  # engine-facing view of an mm-dtype tile
        return ap if mm_bf16 else ap.bitcast(f32)

    with tile.TileContext(nc) as tc, ExitStack() as ctx:
        consts = ctx.enter_context(tc.tile_pool(name="consts", bufs=1))
        xt_pool = ctx.enter_context(tc.tile_pool(name="xt", bufs=3))
        h1_pool = ctx.enter_context(tc.tile_pool(name="h1", bufs=2))
        h2_pool = ctx.enter_context(tc.tile_pool(name="h2", bufs=2))
        sq_pool = ctx.enter_context(tc.tile_pool(name="sq", bufs=1))
        hs_pool = ctx.enter_context(tc.tile_pool(name="hs", bufs=2))
        qs_pool = ctx.enter_context(tc.tile_pool(name="qs", bufs=2))
        rv_pool = ctx.enter_context(tc.tile_pool(name="rv", bufs=2))
        ab_pool = ctx.enter_context(tc.tile_pool(name="ab", bufs=2))
        out_pool = ctx.enter_context(tc.tile_pool(name="outp", bufs=2))
        ps_mlp = ctx.enter_context(tc.tile_pool(name="ps_mlp", bufs=3, space="PSUM"))
        ps_head = ctx.enter_context(tc.tile_pool(name="ps_head", bufs=2, space="PSUM"))
        ps_stat = ctx.enter_context(tc.tile_pool(name="ps_stat", bufs=2, space="PSUM"))
        ps_bc = ctx.enter_context(tc.tile_pool(name="ps_bc", bufs=1, space="PSUM"))

        # --- DMAs round-robin over engine queues: a single queue serializes
        # ~0.65us per 128KB transfer, which was the whole startup stall ---
        dma_engines = [nc.sync, nc.scalar, nc.gpsimd]
        dma_rr = [0]

        def dma(out, in_):
            eng = dma_engines[dma_rr[0] % len(dma_engines)]
            dma_rr[0] += 1
            eng.dma_start(out=out, in_=in_)

        # --- prefetch the first blocks' inputs so the PE can start before
        # the bulk of the constant DMAs land ---
        xt_prefetch = {}

        def load_xt(c0, nb):
            xt_t = xt_pool.tile([P, KL * NB], mmdt, tag="xt")
            for k in range(KL):
                dma(
                    xt_t[:, k * NB : k * NB + nb],
                    xt[k * P : (k + 1) * P, c0 : c0 + nb],
                )
            return xt_t

        for bi in range(min(3, len(blocks))):
            _, _c0, _nb = blocks[bi]
            xt_prefetch[bi] = load_xt(_c0, _nb)

        # --- constants / weights, loaded once, ordered by first use: w1
        # m-halves, then block 0's head weights, then w2 m-halves, then the
        # remaining heads ---
        w1_sb = consts.tile([P, KL * HIDDEN], mmdt)
        for half in range(2):
            for k in range(KL):
                dma(
                    w1_sb[
                        :,
                        k * HIDDEN + half * (HIDDEN // 2) : k * HIDDEN
                        + (half + 1) * (HIDDEN // 2),
                    ],
                    w1d[k * P : (k + 1) * P,
                        half * (HIDDEN // 2) : (half + 1) * (HIDDEN // 2)],
                )
        b1_sb = consts.tile([P, MH], f32)
        nc.sync.dma_start(out=b1_sb[:], in_=b1d[:])
        whp_sb = consts.tile([P, TYPES * KH * OUT], mmdt)
        t0_first = blocks[0][0] if blocks else 0
        type_order = [t0_first] + [t for t in range(TYPES) if t != t0_first]

        def load_whp(t):
            for k in range(KH):
                dma(
                    whp_sb[:, (t * KH + k) * OUT : (t * KH + k + 1) * OUT],
                    whpd[t, k * P : (k + 1) * P, :],
                )

        w2_sb = consts.tile([P, KH * HIDDEN], mmdt)

        def load_w2_half(half):
            for k in range(KH):
                dma(
                    w2_sb[
                        :,
                        k * HIDDEN + half * (HIDDEN // 2) : k * HIDDEN
                        + (half + 1) * (HIDDEN // 2),
                    ],
                    w2d[k * P : (k + 1) * P,
                        half * (HIDDEN // 2) : (half + 1) * (HIDDEN // 2)],
                )

        load_w2_half(0)
        load_w2_half(1)
        load_whp(type_order[0])
        for t in type_order[1:]:
            load_whp(t)
        b2_sb = consts.tile([P, MH], f32)
        nc.sync.dma_start(out=b2_sb[:], in_=b2d[:])
        c1_sb = consts.tile([1, TYPES * OUT], mmdt)
        c2_sb = consts.tile([1, TYPES * OUT], mmdt)
        for t in range(TYPES):
            nc.sync.dma_start(out=c1_sb[:, t * OUT : (t + 1) * OUT], in_=c1d[t])
            nc.sync.dma_start(out=c2_sb[:, t * OUT : (t + 1) * OUT], in_=c2d[t])
        ones_col_bf = consts.tile([P, 1], bf16)
        nc.vector.memset(ones_col_bf[:], 1.0)
        ones_row = consts.tile([1, P], f32r)  # lhsT for partition broadcast
        nc.sync.dma_start(out=ones_row[:], in_=orowd[:])
        eps_ap = consts.tile([1, 1], f32)
        nc.vector.memset(eps_ap[:], LN_EPS)
        act_warm = consts.tile([1, 1], f32)
        nc.scalar.activation(act_warm[:], eps_ap[:], AF.Sqrt)

        # --- per-block pipeline (software-pipelined: the LN-dependent PE ops
        # of block b are emitted mid-block b+1 so the PE never waits on the
        # ACT/DVE stats chain and the HAM clock stays warm) ---

        def emit_tail(t, c0, nb, ph_list, negmu, sv, rsig):
            # rank-1 corrections close the head psum accumulation groups
            for mc in range(MO):
                ph = ph_list[mc]
                nc.tensor.matmul(
                    ph[:, :nb],
                    lhsT=c2_sb[:, t * OUT + mc * P : t * OUT + (mc + 1) * P],
                    rhs=negmu[:, :nb],
                    start=False,
                    stop=not use_c1,
                )
                if use_c1:
                    nc.tensor.matmul(
                        ph[:, :nb],
                        lhsT=c1_sb[:, t * OUT + mc * P : t * OUT + (mc + 1) * P],
                        rhs=sv[:, :nb],
                        start=False,
                        stop=True,
                    )
            # broadcast rsig across partitions (K=1 matmul), stash in SBUF
            ps_a = ps_bc.tile([P, NB], f32, tag="bc")
            nc.tensor.matmul(
                ps_a[:, :nb], lhsT=ones_row[:], rhs=rsig[:, :nb],
                start=True, stop=True,
            )
            a_sb = ab_pool.tile([P, NB], f32, tag="a")
            nc.scalar.activation(a_sb[:, :nb], ps_a[:, :nb], AF.Identity)
            out_sb = out_pool.tile([P, MO * NB], f32, tag="out")
            for mc in range(MO):
                nc.vector.tensor_mul(
                    out_sb[:, mc * NB : mc * NB + nb], ph_list[mc][:, :nb],
                    a_sb[:, :nb],
                )
                nc.sync.dma_start(
                    out=outd[mc * P : (mc + 1) * P, c0 : c0 + nb],
                    in_=out_sb[:, mc * NB : mc * NB + nb],
                )

        pending = []
        TAIL_DEPTH = 1
        for bi, (t, c0, nb) in enumerate(blocks):
            xt_t = xt_prefetch.pop(bi, None)
            if xt_t is None:
                xt_t = load_xt(c0, nb)

            # stage 1: h1^T = relu(W1^T x + b1)   [HIDDEN, nb]
            h1_t = h1_pool.tile([P, MH * NB], mmdt, tag="h1")
            for m in range(MH):
                ps = ps_mlp.tile([P, NB], f32, tag="ps_mlp")
                for k in range(KL):
                    nc.tensor.matmul(
                        ps[:, :nb],
                        lhsT=w1_sb[:, k * HIDDEN + m * P : k * HIDDEN + (m + 1) * P],
                        rhs=xt_t[:, k * NB : k * NB + nb],
                        start=(k == 0),
                        stop=(k == KL - 1),
                    )
                nc.vector.tensor_scalar(
                    h1_t[:, m * NB : m * NB + nb],
                    ps[:, :nb],
                    b1_sb[:, m : m + 1],
                    0.0,
                    op0=mybir.AluOpType.add,
                    op1=mybir.AluOpType.max,
                )

            # deferred LN tails of earlier blocks slot in here: their PE
            # inputs (negmu/sv/rsig) became ready while the blocks in between
            # ran, so the rank-1 matmuls below never stall the PE
            if len(pending) >= TAIL_DEPTH:
                pending.pop(0)()

            # stage 2: h2^T = W2^T h1 + b2; squares ride along per chunk
            h2_t = h2_pool.tile([P, MH * NB], mmdt, tag="h2")
            sq_t = sq_pool.tile([P, MH * NB], bf16, tag="sq")
            for m in range(MH):
                ps = ps_mlp.tile([P, NB], f32, tag="ps_mlp")
                for k in range(KH):
                    nc.tensor.matmul(
                        ps[:, :nb],
                        lhsT=w2_sb[:, k * HIDDEN + m * P : k * HIDDEN + (m + 1) * P],
                        rhs=h1_t[:, k * NB : k * NB + nb],
                        start=(k == 0),
                        stop=(k == KH - 1),
                    )
                nc.scalar.activation(
                    h2_t[:, m * NB : m * NB + nb],
                    ps[:, :nb],
                    AF.Identity,
                    bias=b2_sb[:, m : m + 1],
                )
                nc.vector.tensor_mul(
                    sq_t[:, m * NB : m * NB + nb],
                    cv(h2_t[:, m * NB : m * NB + nb]),
                    cv(h2_t[:, m * NB : m * NB + nb]),
                )

            # head main matmuls: only need h2, so they keep the PE hot while
            # the stats chain below runs on ACT/DVE
            ph_list = []
            for mc in range(MO):
                ph = ps_head.tile([P, NB], f32, tag="head")
                for k in range(KH):
                    nc.tensor.matmul(
                        ph[:, :nb],
                        lhsT=whp_sb[
                            :,
                            (t * KH + k) * OUT + mc * P : (t * KH + k) * OUT
                            + (mc + 1) * P,
                        ],
                        rhs=h2_t[:, k * NB : k * NB + nb],
                        start=(k == 0),
                        stop=False,
                    )
                ph_list.append(ph)

            # LN stats: pairwise-add tile pairs on DVE, then column sums of
            # the halved sets via ones-matmul (keeps PE work low)
            hs_t = hs_pool.tile([P, (MH // 2) * NB], bf16, tag="hs")
            qs_t = qs_pool.tile([P, (MH // 2) * NB], bf16, tag="qs")
            for k in range(MH // 2):
                nc.vector.tensor_add(
                    hs_t[:, k * NB : k * NB + nb],
                    cv(h2_t[:, 2 * k * NB : 2 * k * NB + nb]),
                    cv(h2_t[:, (2 * k + 1) * NB : (2 * k + 1) * NB + nb]),
                )
                nc.vector.tensor_add(
                    qs_t[:, k * NB : k * NB + nb],
                    sq_t[:, 2 * k * NB : 2 * k * NB + nb],
                    sq_t[:, (2 * k + 1) * NB : (2 * k + 1) * NB + nb],
                )
            for k in range(MH // 4):
                nc.vector.tensor_add(
                    hs_t[:, k * NB : k * NB + nb],
                    hs_t[:, 2 * k * NB : 2 * k * NB + nb],
                    hs_t[:, (2 * k + 1) * NB : (2 * k + 1) * NB + nb],
                )
                nc.vector.tensor_add(
                    qs_t[:, k * NB : k * NB + nb],
                    qs_t[:, 2 * k * NB : 2 * k * NB + nb],
                    qs_t[:, (2 * k + 1) * NB : (2 * k + 1) * NB + nb],
                )
            nc.vector.tensor_add(
                hs_t[:, :nb], hs_t[:, :nb], hs_t[:, NB : NB + nb]
            )
            nc.vector.tensor_add(
                qs_t[:, :nb], qs_t[:, :nb], qs_t[:, NB : NB + nb]
            )
            ps_s = ps_stat.tile([1, NB], f32, tag="stat")
            nc.tensor.matmul(
                ps_s[:, :nb], lhsT=ones_col_bf[:], rhs=hs_t[:, :nb],
                start=True, stop=True,
            )
            ps_q = ps_stat.tile([1, NB], f32, tag="stat")
            nc.tensor.matmul(
                ps_q[:, :nb], lhsT=ones_col_bf[:], rhs=qs_t[:, :nb],
                start=True, stop=True,
            )

            negmu = rv_pool.tile([1, NB], mmdt, tag="negmu")
            nc.scalar.activation(
                negmu[:, :nb], ps_s[:, :nb], AF.Identity, scale=-1.0 / HIDDEN
            )
            musq = rv_pool.tile([1, NB], f32, tag="musq")
            nc.scalar.activation(
                musq[:, :nb], ps_s[:, :nb], AF.Square, scale=1.0 / HIDDEN
            )
            varv = rv_pool.tile([1, NB], f32, tag="varv")
            nc.scalar.activation(
                varv[:, :nb], ps_q[:, :nb], AF.Identity, scale=1.0 / HIDDEN
            )
            nc.vector.tensor_sub(varv[:, :nb], varv[:, :nb], musq[:, :nb])
            svf = rv_pool.tile([1, NB], f32, tag="svf")  # sqrt(var + eps)
            nc.scalar.activation(svf[:, :nb], varv[:, :nb], AF.Sqrt, bias=eps_ap[:])
            if use_c1:
                sv = rv_pool.tile([1, NB], mmdt, tag="sv")
                nc.scalar.activation(sv[:, :nb], varv[:, :nb], AF.Sqrt, bias=eps_ap[:])
            else:
                sv = None
            rsf = rv_pool.tile([1, NB], f32, tag="rsf")
            nc.vector.reciprocal_approx_fast(rsf[:, :nb], svf[:, :nb])
            rsig = rv_pool.tile([1, NB], f32r, tag="rsig")
            nc.scalar.activation(rsig[:, :nb], rsf[:, :nb], AF.Identity)

            import functools

            pending.append(functools.partial(
                emit_tail, t, c0, nb, ph_list, negmu, sv, rsig
            ))

        for p in pending:
            p()

    nc.compile()
    return nc


def plan(node_types, pad_odd=True):
    """Host-side layout plan shared by all cores.

    Returns (blocks, R, caps, idx_by_type) where idx_by_type[t][c] is the array
    of original row indices of type t assigned to core c.
    """
    node_types = np.asarray(node_types)
    counts = np.bincount(node_types, minlength=TYPES)
    caps = []  # per-core column capacity for each type (multiple of P)
    idx_by_type = []
    order = np.argsort(node_types, kind="stable")
    starts = np.concatenate([[0], np.cumsum(counts)])
    for tt in range(TYPES):
        tiles = int(-(-counts[tt] // (N_CORES * P)))  # ceil to 128-row tiles/core
        if pad_odd and tiles % 4 == 1:
            # a lone 128-col block runs f32r at 1/4 rate - same cost as 2 cols
            tiles += 1
        cap = tiles * P
        caps.append(cap)
        idx_t = order[starts[tt] : starts[tt + 1]]
        base, rem = divmod(int(counts[tt]), N_CORES)
        parts, o = [], 0
        for c in range(N_CORES):
            n = base + (1 if c < rem else 0)
            parts.append(idx_t[o : o + n])
            o += n
        idx_by_type.append(parts)
    R = sum(caps)
    blocks = []
    off = 0
    for tt in range(TYPES):
        tiles = caps[tt] // P
        j = 0
        while j < tiles:
            nt = min(NB // P, tiles - j)
            blocks.append((tt, off + j * P, nt * P))
            j += nt
        off += caps[tt]
    return blocks, R, caps, idx_by_type


def _tf32(x):
    """Round fp32 to TF32 (10-bit mantissa, round-to-nearest-even)."""
    u = np.ascontiguousarray(x, dtype=np.float32).view(np.uint32).copy()
    lsb = (u >> np.uint32(13)) & np.uint32(1)
    u += np.uint32(0x0FFF) + lsb
    u &= np.uint32(0xFFFFE000)
    return u.view(np.float32)


def prep_inputs(node_latent, w1, b1, w2, b2, ln_gamma, ln_beta, head_w, head_b,
                caps, idx_by_type, mm_bf16=False):
    """Build the 8 per-core input maps."""
    if mm_bf16:
        import ml_dtypes

        cast = lambda a: np.asarray(a, dtype=np.float32).astype(ml_dtypes.bfloat16)
    else:
        cast = _tf32
    whp = cast(ln_gamma[:, None] * head_w)  # [T, H, OUT]
    c1 = cast(np.asarray(ln_beta @ head_w + head_b)).reshape(TYPES, 1, OUT)
    c2 = cast(np.asarray(ln_gamma @ head_w)).reshape(TYPES, 1, OUT)
    b1r = np.ascontiguousarray(b1.reshape(MH, P).T).astype(np.float32)
    b2r = np.ascontiguousarray(b2.reshape(MH, P).T).astype(np.float32)
    R = sum(caps)
    in_maps = []
    for c in range(N_CORES):
        xc = np.zeros((R, LATENT), np.float32)
        off = 0
        for tt in range(TYPES):
            idx = idx_by_type[tt][c]
            xc[off : off + len(idx)] = node_latent[idx]
            off += caps[tt]
        in_maps.append(
            {
                "xt": cast(xc.T),
                "w1": cast(w1),
                "w2": cast(w2),
                "b1r": b1r,
                "b2r": b2r,
                "whp": whp,
                "c1": c1,
                "c2": c2,
                "orow": np.ones((1, P), np.float32),
            }
        )
    return in_maps


def unpack_outputs(results, caps, idx_by_type, n_rows):
    out = np.empty((n_rows, OUT), np.float32)
    for c in range(N_CORES):
        oc = results[c]["out"]  # [OUT, R]
        off = 0
        for tt in range(TYPES):
            idx = idx_by_type[tt][c]
            out[idx] = oc[:, off : off + len(idx)].T
            off += caps[tt]
    return out


MM_BF16 = True


def kernel(node_latent, node_types, w1, b1, w2, b2, ln_gamma, ln_beta, head_w, head_b):
    from concourse.bass_utils import run_bass_kernel_spmd

    node_latent = np.asarray(node_latent, dtype=np.float32)
    node_types = np.asarray(node_types)
    blocks, R, caps, idx_by_type = plan(node_types, pad_odd=not MM_BF16)
    use_c1 = bool(np.any(np.asarray(ln_beta @ head_w + head_b)))
    nc = build_program(blocks, R, use_c1=use_c1, mm_bf16=MM_BF16)
    in_maps = prep_inputs(
        node_latent, w1, b1, w2, b2, ln_gamma, ln_beta, head_w, head_b,
        caps, idx_by_type, mm_bf16=MM_BF16,
    )
    res = run_bass_kernel_spmd(nc, in_maps, core_ids=list(range(N_CORES)))
    return unpack_outputs(res.results, caps, idx_by_type, node_latent.shape[0])



# revision 8
# speedup vs baseline: 1.2303x; 1.2303x over previous
"""Trainium2 Bass kernel for nn_AdaptiveDecoder (shared MLP + hard-routed type heads).

Strategy:
  * Host: sort nodes by type; split each type's count over 8 cores with minimal
    padding (per-type cap = ceil(count/8) rounded to 4) -> every core sees the
    SAME static layout of type-pure node-column blocks, so the compiled SPMD
    program bakes in the block->head mapping and the device does zero routing.
  * Blocks are 512 node-columns; a type's remainder is split into two ~equal
    blocks (>=256 cols) so no matmul is short enough to become LDWEIGHTS-bound.
  * Device: activations stay transposed ([feature, nodes]) so the three matmul
    stages chain without transposes.  LayerNorm's gamma is folded into the head
    weights on the host; the per-node mean enters via K=1 rank-1 accumulation
    matmuls and the 1/sigma factor via a K=1 broadcast matmul.  Those three
    K=1 matmuls are packed into ONE PE slot via row-group tiling (lhsT/rhs at
    partitions 0/32/64 -> concurrent 32-row sub-arrays).
  * The LN stats column-sum matmuls emit their sums at partitions {0,32,64}
    directly (ones at lhsT cols 0/32/64), so the rank-1 rhs vectors are born on
    the row-group partitions they are consumed at -- no partition moves.
  * All DRAM inputs are pre-tiled on the host into their exact SBUF layouts so
    every load is one dma_start with multi-KB contiguous rows (fast startup).
  * Matmuls run bf16 (same PE rate as f32r, half the SBUF traffic).
"""

import sys

sys.path.insert(0, "/opt/trn_rl_repo")

from contextlib import ExitStack

import numpy as np

N_CORES = 8
LATENT, HIDDEN, OUT, TYPES = 512, 1024, 256, 3
P = 128
NB = 512  # node columns per block (PSUM f32 bank limit)
KL = LATENT // P  # 4 k-tiles, stage 1
KH = HIDDEN // P  # 8 k-tiles, stage 2 / head
MH = HIDDEN // P  # 8 m-chunks of hidden
MO = OUT // P  # 2 m-chunks of head output
LN_EPS = 1e-5
MM_BF16 = True


def _caps_from_counts(counts):
    caps = []
    for tt in range(TYPES):
        cap = -(-int(counts[tt]) // N_CORES)  # ceil
        cap = -(-cap // 4) * 4  # round to 4 cols (keeps DMA rows 8B-aligned)
        caps.append(cap)
    return caps


def _blocks_from_caps(caps):
    """Type-pure blocks tiling [0, R). Remainders split so blocks stay >=256."""
    blocks = []
    off = 0
    for tt in range(TYPES):
        cols = caps[tt]
        j = 0
        while j < cols:
            rem = cols - j
            if rem >= 2 * NB:
                nb = NB
            elif rem > NB:
                nb = (rem + 1) // 2
                nb = -(-nb // 4) * 4
            else:
                nb = rem
            blocks.append((tt, off + j, nb))
            j += nb
        off += cols
    return blocks


def plan(node_types, pad_odd=True):
    """Host-side layout plan shared by all cores.

    Returns (blocks, R, caps, idx_by_type) where idx_by_type[t][c] is the array
    of original row indices of type t assigned to core c.
    """
    node_types = np.asarray(node_types)
    counts = np.bincount(node_types, minlength=TYPES)
    caps = _caps_from_counts(counts)
    idx_by_type = []
    order = np.argsort(node_types, kind="stable")
    starts = np.concatenate([[0], np.cumsum(counts)])
    for tt in range(TYPES):
        idx_t = order[starts[tt] : starts[tt + 1]]
        base, rem = divmod(int(counts[tt]), N_CORES)
        parts, o = [], 0
        for c in range(N_CORES):
            n = base + (1 if c < rem else 0)
            parts.append(idx_t[o : o + n])
            o += n
        idx_by_type.append(parts)
    R = sum(caps)
    blocks = _blocks_from_caps(caps)
    return blocks, R, caps, idx_by_type


def build_program(blocks, R, use_c1=True, mm_bf16=True):
    """blocks: list of (type_idx, col_offset, n_cols); R: node columns per core."""
    import concourse.mybir as mybir
    import concourse.tile as tile
    from concourse import bacc

    dt = mybir.dt
    f32, f32r, bf16 = dt.float32, dt.float32r, dt.bfloat16
    mmdt = bf16 if mm_bf16 else f32r
    AF = mybir.ActivationFunctionType

    nc = bacc.Bacc("TRN2", target_bir_lowering=False, debug=False, num_devices=N_CORES)

    xtd = nc.dram_tensor("xtp", [P, KL * R], mmdt, kind="ExternalInput").ap()
    w1d = nc.dram_tensor("w1p", [P, KL * HIDDEN], mmdt, kind="ExternalInput").ap()
    w2d = nc.dram_tensor("w2p", [P, KH * HIDDEN], mmdt, kind="ExternalInput").ap()
    whpd = nc.dram_tensor("whpp", [P, TYPES * KH * OUT], mmdt, kind="ExternalInput").ap()
    b1d = nc.dram_tensor("b1r", [P, MH], f32, kind="ExternalInput").ap()
    b2d = nc.dram_tensor("b2r", [P, MH], f32, kind="ExternalInput").ap()
    c1d = nc.dram_tensor("c1r", [1, TYPES * OUT], mmdt, kind="ExternalInput").ap()
    c2d = nc.dram_tensor("c2r", [1, TYPES * OUT], mmdt, kind="ExternalInput").ap()
    orowd = nc.dram_tensor("orow", [1, P], f32r, kind="ExternalInput").ap()
    outd = nc.dram_tensor("out", [OUT, R], f32, kind="ExternalOutput").ap()

    def cv(ap):  # engine-facing view of an mm-dtype tile
        return ap if mm_bf16 else ap.bitcast(f32)

    with tile.TileContext(nc) as tc, ExitStack() as ctx:
        consts = ctx.enter_context(tc.tile_pool(name="consts", bufs=1))
        xt_pool = ctx.enter_context(tc.tile_pool(name="xt", bufs=3))
        h1_pool = ctx.enter_context(tc.tile_pool(name="h1", bufs=2))
        h2_pool = ctx.enter_context(tc.tile_pool(name="h2", bufs=2))
        sq_pool = ctx.enter_context(tc.tile_pool(name="sq", bufs=1))
        hs_pool = ctx.enter_context(tc.tile_pool(name="hs", bufs=2))
        qs_pool = ctx.enter_context(tc.tile_pool(name="qs", bufs=2))
        rv_pool = ctx.enter_context(tc.tile_pool(name="rv", bufs=2))
        ab_pool = ctx.enter_context(tc.tile_pool(name="ab", bufs=2))
        out_pool = ctx.enter_context(tc.tile_pool(name="outp", bufs=2))
        ps_mlp = ctx.enter_context(tc.tile_pool(name="ps_mlp", bufs=3, space="PSUM"))
        ps_head = ctx.enter_context(tc.tile_pool(name="ps_head", bufs=2, space="PSUM"))
        ps_stat = ctx.enter_context(tc.tile_pool(name="ps_stat", bufs=2, space="PSUM"))
        ps_bc = ctx.enter_context(tc.tile_pool(name="ps_bc", bufs=1, space="PSUM"))

        dma_engines = [nc.sync, nc.scalar, nc.gpsimd]
        dma_rr = [0]

        def dma(out, in_):
            eng = dma_engines[dma_rr[0] % len(dma_engines)]
            dma_rr[0] += 1
            eng.dma_start(out=out, in_=in_)

        def load_xt(c0, nb, eng=None):
            xt_t = xt_pool.tile([P, KL * NB], mmdt, tag="xt")
            if eng is None:
                dma(xt_t[:, : KL * nb], xtd[:, KL * c0 : KL * (c0 + nb)])
            else:
                eng.dma_start(out=xt_t[:, : KL * nb], in_=xtd[:, KL * c0 : KL * (c0 + nb)])
            return xt_t

        # --- startup: block 0's input + the first-needed weights go first, on
        # separate queues, as single contiguous-row transfers ---
        xt_prefetch = {}
        xt_prefetch[0] = load_xt(blocks[0][1], blocks[0][2], eng=nc.sync)
        w1_sb = consts.tile([P, KL * HIDDEN], mmdt)
        for half in range(2):  # halves so stage-1 m0-3 can start before m4-7 land
            nc.scalar.dma_start(
                out=w1_sb[:, half * (KL * HIDDEN) // 2 : (half + 1) * (KL * HIDDEN) // 2],
                in_=w1d[:, half * (KL * HIDDEN) // 2 : (half + 1) * (KL * HIDDEN) // 2],
            )
        b1_sb = consts.tile([P, MH], f32)
        nc.gpsimd.dma_start(out=b1_sb[:], in_=b1d[:])
        b2_sb = consts.tile([P, MH], f32)
        nc.gpsimd.dma_start(out=b2_sb[:], in_=b2d[:])
        # rank-1 constants live at partitions {32,64} = the row groups that use them
        c2t = consts.tile([65, TYPES * OUT], mmdt)
        nc.gpsimd.dma_start(out=c2t[32:33, :], in_=c2d[:])
        nc.gpsimd.dma_start(out=c2t[64:65, :], in_=c2d[:])
        c1t = consts.tile([65, TYPES * OUT], mmdt)
        if use_c1:
            nc.gpsimd.dma_start(out=c1t[32:33, :], in_=c1d[:])
            nc.gpsimd.dma_start(out=c1t[64:65, :], in_=c1d[:])
        onesr = consts.tile([1, P], f32r)  # lhsT for the rsig broadcast (row group 0)
        nc.gpsimd.dma_start(out=onesr[:], in_=orowd[:])

        if len(blocks) > 1:
            xt_prefetch[1] = load_xt(blocks[1][1], blocks[1][2], eng=nc.sync)

        whp_sb = consts.tile([P, TYPES * KH * OUT], mmdt)
        t0_first = blocks[0][0] if blocks else 0
        type_order = [t0_first] + [t for t in range(TYPES) if t != t0_first]
        nc.scalar.dma_start(
            out=whp_sb[:, t0_first * KH * OUT : (t0_first + 1) * KH * OUT],
            in_=whpd[:, t0_first * KH * OUT : (t0_first + 1) * KH * OUT],
        )
        w2_sb = consts.tile([P, KH * HIDDEN], mmdt)
        nc.gpsimd.dma_start(out=w2_sb[:], in_=w2d[:])
        for t in type_order[1:]:
            nc.scalar.dma_start(
                out=whp_sb[:, t * KH * OUT : (t + 1) * KH * OUT],
                in_=whpd[:, t * KH * OUT : (t + 1) * KH * OUT],
            )
        if len(blocks) > 2:
            xt_prefetch[2] = load_xt(blocks[2][1], blocks[2][2], eng=nc.sync)

        # ones at lhsT cols {0,32,64}: the stats matmul emits its column sums
        # at partitions 0/32/64 simultaneously
        ones65 = consts.tile([P, 65], bf16)
        nc.vector.memset(ones65[:], 0.0)
        for cc in (0, 32, 64):
            nc.vector.memset(ones65[:, cc : cc + 1], 1.0)
        eps65 = consts.tile([65, 1], f32)
        nc.vector.memset(eps65[:], LN_EPS)
        act_warm = consts.tile([1, 1], f32)
        nc.scalar.activation(act_warm[:], eps65[0:1, :], AF.Sqrt)

        # --- per-block pipeline (software-pipelined: the LN-dependent PE ops
        # of block b are emitted mid-block b+1 so the PE never waits on the
        # ACT/DVE stats chain) ---

        def emit_tail(t, c0, nb, ph_list, negmu_t, sv_t, rsig_t):
            # rank-1 corrections + rsig broadcast, packed into one PE slot via
            # row groups 0/1/2 (lhsT+rhs at partitions 0/32/64, distinct banks)
            ps_a = ps_bc.tile([P, NB], f32, tag="bc")
            nc.tensor.matmul(
                ps_a[:, :nb], lhsT=onesr[:], rhs=rsig_t[0:1, :nb],
                start=True, stop=True,
            )
            nc.tensor.matmul(
                ph_list[0][:, :nb],
                lhsT=c2t[32:33, t * OUT : t * OUT + P],
                rhs=negmu_t[32:33, :nb],
                start=False,
                stop=not use_c1,
            )
            nc.tensor.matmul(
                ph_list[1][:, :nb],
                lhsT=c2t[64:65, t * OUT + P : t * OUT + 2 * P],
                rhs=negmu_t[64:65, :nb],
                start=False,
                stop=not use_c1,
            )
            if use_c1:
                nc.tensor.matmul(
                    ph_list[0][:, :nb],
                    lhsT=c1t[32:33, t * OUT : t * OUT + P],
                    rhs=sv_t[32:33, :nb],
                    start=False,
                    stop=True,
                )
                nc.tensor.matmul(
                    ph_list[1][:, :nb],
                    lhsT=c1t[64:65, t * OUT + P : t * OUT + 2 * P],
                    rhs=sv_t[64:65, :nb],
                    start=False,
                    stop=True,
                )
            a_sb = ab_pool.tile([P, NB], f32, tag="a")
            nc.scalar.activation(a_sb[:, :nb], ps_a[:, :nb], AF.Identity)
            out_sb = out_pool.tile([P, MO * NB], f32, tag="out")
            for mc in range(MO):
                nc.vector.tensor_mul(
                    out_sb[:, mc * NB : mc * NB + nb], ph_list[mc][:, :nb],
                    a_sb[:, :nb],
                )
                nc.sync.dma_start(
                    out=outd[mc * P : (mc + 1) * P, c0 : c0 + nb],
                    in_=out_sb[:, mc * NB : mc * NB + nb],
                )

        import functools

        pending = []
        for bi, (t, c0, nb) in enumerate(blocks):
            xt_t = xt_prefetch.pop(bi, None)
            if xt_t is None:
                xt_t = load_xt(c0, nb)

            # stage 1: h1^T = relu(W1^T x + b1)   [HIDDEN, nb]
            h1_t = h1_pool.tile([P, MH * NB], mmdt, tag="h1")
            for m in range(MH):
                ps = ps_mlp.tile([P, NB], f32, tag="ps_mlp")
                for k in range(KL):
                    nc.tensor.matmul(
                        ps[:, :nb],
                        lhsT=w1_sb[:, k * HIDDEN + m * P : k * HIDDEN + (m + 1) * P],
                        rhs=xt_t[:, k * nb : (k + 1) * nb],
                        start=(k == 0),
                        stop=(k == KL - 1),
                    )
                nc.vector.tensor_scalar(
                    h1_t[:, m * NB : m * NB + nb],
                    ps[:, :nb],
                    b1_sb[:, m : m + 1],
                    0.0,
                    op0=mybir.AluOpType.add,
                    op1=mybir.AluOpType.max,
                )

            # deferred LN tail of the previous block slots in here: its PE
            # inputs (negmu/sv/rsig) became ready while this block's stage 1
            # ran, so the packed rank-1 slot never stalls the PE
            if pending:
                pending.pop(0)()

            # stage 2: h2^T = W2^T h1 + b2; squares ride along per chunk
            h2_t = h2_pool.tile([P, MH * NB], mmdt, tag="h2")
            sq_t = sq_pool.tile([P, MH * NB], bf16, tag="sq")
            for m in range(MH):
                ps = ps_mlp.tile([P, NB], f32, tag="ps_mlp")
                for k in range(KH):
                    nc.tensor.matmul(
                        ps[:, :nb],
                        lhsT=w2_sb[:, k * HIDDEN + m * P : k * HIDDEN + (m + 1) * P],
                        rhs=h1_t[:, k * NB : k * NB + nb],
                        start=(k == 0),
                        stop=(k == KH - 1),
                    )
                nc.scalar.activation(
                    h2_t[:, m * NB : m * NB + nb],
                    ps[:, :nb],
                    AF.Identity,
                    bias=b2_sb[:, m : m + 1],
                )
                nc.vector.tensor_mul(
                    sq_t[:, m * NB : m * NB + nb],
                    cv(h2_t[:, m * NB : m * NB + nb]),
                    cv(h2_t[:, m * NB : m * NB + nb]),
                )

            # head main matmuls: only need h2, so they keep the PE hot while
            # the stats chain below runs on ACT/DVE
            ph_list = []
            for mc in range(MO):
                ph = ps_head.tile([P, NB], f32, tag="head")
                for k in range(KH):
                    nc.tensor.matmul(
                        ph[:, :nb],
                        lhsT=whp_sb[
                            :,
                            (t * KH + k) * OUT + mc * P : (t * KH + k) * OUT
                            + (mc + 1) * P,
                        ],
                        rhs=h2_t[:, k * NB : k * NB + nb],
                        start=(k == 0),
                        stop=False,
                    )
                ph_list.append(ph)

            # LN stats: pairwise-add tile pairs on DVE, then column sums of
            # the halved sets via ones-matmuls that emit at partitions 0/32/64
            hs_t = hs_pool.tile([P, (MH // 2) * NB], bf16, tag="hs")
            qs_t = qs_pool.tile([P, (MH // 2) * NB], bf16, tag="qs")
            for k in range(MH // 2):
                nc.vector.tensor_add(
                    hs_t[:, k * NB : k * NB + nb],
                    cv(h2_t[:, 2 * k * NB : 2 * k * NB + nb]),
                    cv(h2_t[:, (2 * k + 1) * NB : (2 * k + 1) * NB + nb]),
                )
                nc.vector.tensor_add(
                    qs_t[:, k * NB : k * NB + nb],
                    sq_t[:, 2 * k * NB : 2 * k * NB + nb],
                    sq_t[:, (2 * k + 1) * NB : (2 * k + 1) * NB + nb],
                )
            for k in range(MH // 4):
                nc.vector.tensor_add(
                    hs_t[:, k * NB : k * NB + nb],
                    hs_t[:, 2 * k * NB : 2 * k * NB + nb],
                    hs_t[:, (2 * k + 1) * NB : (2 * k + 1) * NB + nb],
                )
                nc.vector.tensor_add(
                    qs_t[:, k * NB : k * NB + nb],
                    qs_t[:, 2 * k * NB : 2 * k * NB + nb],
                    qs_t[:, (2 * k + 1) * NB : (2 * k + 1) * NB + nb],
                )
            nc.vector.tensor_add(
                hs_t[:, :nb], hs_t[:, :nb], hs_t[:, NB : NB + nb]
            )
            nc.vector.tensor_add(
                qs_t[:, :nb], qs_t[:, :nb], qs_t[:, NB : NB + nb]
            )
            ps_s = ps_stat.tile([65, NB], f32, tag="stat")
            nc.tensor.matmul(
                ps_s[:, :nb], lhsT=ones65[:], rhs=hs_t[:, :nb],
                start=True, stop=True,
            )
            ps_q = ps_stat.tile([65, NB], f32, tag="stat")
            nc.tensor.matmul(
                ps_q[:, :nb], lhsT=ones65[:], rhs=qs_t[:, :nb],
                start=True, stop=True,
            )

            # negmu at partitions {32,64} (rank-1 rhs), var chain at {0}
            negmu_t = rv_pool.tile([65, NB], mmdt, tag="negmu")
            nc.scalar.activation(
                negmu_t[:, :nb], ps_s[:, :nb], AF.Identity, scale=-1.0 / HIDDEN
            )
            musq_t = rv_pool.tile([65, NB], f32, tag="musq")
            nc.scalar.activation(
                musq_t[:, :nb], ps_s[:, :nb], AF.Square, scale=1.0 / HIDDEN
            )
            varv_t = rv_pool.tile([65, NB], f32, tag="varv")
            nc.scalar.activation(
                varv_t[:, :nb], ps_q[:, :nb], AF.Identity, scale=1.0 / HIDDEN
            )
            nc.vector.tensor_sub(varv_t[:, :nb], varv_t[:, :nb], musq_t[:, :nb])
            if use_c1:
                sv_t = rv_pool.tile([65, NB], mmdt, tag="sv")
                nc.scalar.activation(
                    sv_t[:, :nb], varv_t[:, :nb], AF.Sqrt, bias=eps65[:, :]
                )
            else:
                sv_t = None
            svf_t = rv_pool.tile([1, NB], f32, tag="svf")
            nc.scalar.activation(
                svf_t[:, :nb], varv_t[0:1, :nb], AF.Sqrt, bias=eps65[0:1, :]
            )
            rsf_t = rv_pool.tile([1, NB], f32, tag="rsf")
            nc.vector.reciprocal_approx_fast(rsf_t[:, :nb], svf_t[:, :nb])
            rsig_t = rv_pool.tile([1, NB], f32r, tag="rsig")
            nc.scalar.activation(rsig_t[:, :nb], rsf_t[:, :nb], AF.Identity)

            pending.append(functools.partial(
                emit_tail, t, c0, nb, ph_list, negmu_t, sv_t, rsig_t
            ))

        for pf in pending:
            pf()

    nc.compile()
    return nc


def _tf32(x):
    """Round fp32 to TF32 (10-bit mantissa, round-to-nearest-even)."""
    u = np.ascontiguousarray(x, dtype=np.float32).view(np.uint32).copy()
    lsb = (u >> np.uint32(13)) & np.uint32(1)
    u += np.uint32(0x0FFF) + lsb
    u &= np.uint32(0xFFFFE000)
    return u.view(np.float32)


def _tile_cols(a, kt):
    """[kt*P, C] -> [P, kt*C] with col index = k*C + c (the SBUF layout)."""
    kp, C = a.shape
    assert kp == kt * P
    return np.ascontiguousarray(
        a.reshape(kt, P, C).transpose(1, 0, 2).reshape(P, kt * C)
    )


def prep_inputs(node_latent, w1, b1, w2, b2, ln_gamma, ln_beta, head_w, head_b,
                caps, idx_by_type, mm_bf16=True):
    """Build the 8 per-core input maps (everything pre-tiled to SBUF layout)."""
    if mm_bf16:
        import ml_dtypes

        cast = lambda a: np.asarray(a, dtype=np.float32).astype(ml_dtypes.bfloat16)
    else:
        cast = _tf32
    whp = np.asarray(ln_gamma)[:, None] * np.asarray(head_w)  # [T, H, OUT]
    whpp = np.concatenate(
        [_tile_cols(cast(whp[t]), KH) for t in range(TYPES)], axis=1
    )  # [P, T*KH*OUT]
    c1 = cast(np.asarray(ln_beta @ head_w + head_b)).reshape(1, TYPES * OUT)
    c2 = cast(np.asarray(ln_gamma @ head_w)).reshape(1, TYPES * OUT)
    w1p = _tile_cols(cast(w1), KL)  # [P, KL*HIDDEN]
    w2p = _tile_cols(cast(w2), KH)  # [P, KH*HIDDEN]
    b1r = np.ascontiguousarray(np.asarray(b1).reshape(MH, P).T).astype(np.float32)
    b2r = np.ascontiguousarray(np.asarray(b2).reshape(MH, P).T).astype(np.float32)
    R = sum(caps)
    blocks = _blocks_from_caps(caps)
    node_latent = np.asarray(node_latent, dtype=np.float32)
    in_maps = []
    for c in range(N_CORES):
        xc = np.zeros((R, LATENT), np.float32)
        off = 0
        for tt in range(TYPES):
            idx = idx_by_type[tt][c]
            xc[off : off + len(idx)] = node_latent[idx]
            off += caps[tt]
        xcb = cast(xc)
        xtp = np.empty((P, KL * R), dtype=xcb.dtype)
        for (_t, c0, nb) in blocks:
            xtp[:, KL * c0 : KL * (c0 + nb)] = (
                xcb[c0 : c0 + nb, :].reshape(nb, KL, P).transpose(2, 1, 0)
                .reshape(P, KL * nb)
            )
        in_maps.append(
            {
                "xtp": xtp,
                "w1p": w1p,
                "w2p": w2p,
                "whpp": whpp,
                "b1r": b1r,
                "b2r": b2r,
                "c1r": c1,
                "c2r": c2,
                "orow": np.ones((1, P), np.float32),
            }
        )
    return in_maps


def unpack_outputs(results, caps, idx_by_type, n_rows):
    out = np.empty((n_rows, OUT), np.float32)
    for c in range(N_CORES):
        oc = results[c]["out"]  # [OUT, R]
        off = 0
        for tt in range(TYPES):
            idx = idx_by_type[tt][c]
            out[idx] = oc[:, off : off + len(idx)].T
            off += caps[tt]
    return out


def kernel(node_latent, node_types, w1, b1, w2, b2, ln_gamma, ln_beta, head_w, head_b):
    from concourse.bass_utils import run_bass_kernel_spmd

    node_latent = np.asarray(node_latent, dtype=np.float32)
    node_types = np.asarray(node_types)
    blocks, R, caps, idx_by_type = plan(node_types)
    use_c1 = bool(np.any(np.asarray(ln_beta @ head_w + head_b)))
    nc = build_program(blocks, R, use_c1=use_c1, mm_bf16=MM_BF16)
    in_maps = prep_inputs(
        node_latent, w1, b1, w2, b2, ln_gamma, ln_beta, head_w, head_b,
        caps, idx_by_type, mm_bf16=MM_BF16,
    )
    res = run_bass_kernel_spmd(nc, in_maps, core_ids=list(range(N_CORES)))
    return unpack_outputs(res.results, caps, idx_by_type, node_latent.shape[0])
